# revision 20
# baseline (speedup 1.0000x reference)
"""TRN2 Bass kernel for nn_AttentionModule (dense transformer attention block).

Reference computation (per sample b, x flattened to [256, 4096]):
    proj = conv_w @ x + conv_b                 [32, 4096]
    q    = (q_w @ proj + q_b).T                [4096, 32]
    k    = k_w @ proj + k_b                    [32, 4096]
    v    = v_w @ proj + v_b                    [256, 4096]
    attn = softmax(q @ k, axis=-1)             [4096(n), 4096(m)]
    out  = gamma * (v @ attn.T) + x            [256, 4096]

Sharding: 8 cores = 4 samples x 2 query-halves (2048 queries each). Each core
redundantly computes proj/k/v for its sample (cheap) and its half of the
queries. No cross-core communication. SPMD: odd cores receive x with the
spatial axis rolled by -2048 so "their" queries sit at columns 0:2048;
attention is permutation-invariant over keys so k/v column order is free.

On-core layout: scores are computed transposed, [m_keys(part), n_queries
(free)], so the exp'd scores chunks are directly usable as matmul weights
(lhsT) for the attn@V contraction over m, and the softmax denominator falls
out of the same matmul via an appended ones-column in the V^T projection
(column 256 of the [33,257] rhs; proj carries a ones-row 32 that also folds
in the v bias). No max-subtraction: exp'd scores are stored in bf16 (no
overflow below e^88); numerator and denominator share the same bf16 rounding
so softmax normalization cancels most of it. The residual is applied in
[n, c] layout against a host-transposed x, and the host transposes the
[128, 16, 256] per-core output back — no on-chip transposes at all.

PSUM layout: two pools. ps_pool = 2 slots x 3 banks [128, 1536] for score
groups (3 m-chunks each); po_pool = 2 slots x 1 bank [128, 512] for the
attnout accumulators and the vT' build. This keeps the attnout accumulation
chain OFF the score-slot rotation: a po allocation never has to wait for an
exp to drain its psum (which used to put the ~2us exp latency on the PE
critical path once per block).

The exp stream is split between ACT (true Exp) and DVE (Schraudolph bit-trick
exp: int16(round(s*128/ln2 + 127*128 - 5.59)) bitcast as bf16, ~3% relative
error that largely cancels between softmax num/denom). Score groups are
emitted between HALF attnout blocks so a group's exp has ~2 block-halves of
PE work to drain before its psum slot is reused.

gamma is folded into v_w/v_b host-side. fp16 feeds the q/k score path.
"""

import numpy as np
from contextlib import ExitStack

import concourse.bass as bass
import concourse.bacc as bacc
import concourse.tile as tile
from concourse import mybir
from concourse.bass_utils import run_bass_kernel_spmd

F32 = mybir.dt.float32
F16 = mybir.dt.float16
BF16 = mybir.dt.bfloat16
I16 = mybir.dt.int16

B, C, H, W = 4, 256, 64, 64
HW = H * W          # 4096 keys (m)
NQ = HW // 2        # 2048 queries per core (n)
C8 = 32             # qk head dim (e) / proj channels (d)
NSUP = 512          # queries per attention super-block
NBLK = 128          # queries per attnout block
MCH = 128           # keys per m-chunk (one lhsT tile)
N_MCH = HW // MCH   # 32 m-chunks
VN = C + 1          # 257: v channels + ones column (softmax denominator)
NB = NQ // NBLK     # 16 attnout blocks

# Schraudolph fast-exp constants (bf16 bit pattern via int16 affine)
SCH_MUL = 184.6650292
SCH_ADD = 16250.41

_CACHED = {}


def build_nc():
    nc = bacc.Bacc("TRN2", target_bir_lowering=False, debug=False)
    d_x16 = nc.dram_tensor("x16", [2, 128, HW], F16, kind="ExternalInput").ap()
    d_xT = nc.dram_tensor("xT", [128, NB, C], F32, kind="ExternalInput").ap()
    d_cwT = nc.dram_tensor("cwT", [2, 128, C8], F16, kind="ExternalInput").ap()
    d_cb = nc.dram_tensor("cb", [C8, 1], F32, kind="ExternalInput").ap()
    # k/q weights carry their bias as row 32, contracted against proj's
    # ones-row — no separate bias op needed.
    d_kwT = nc.dram_tensor("kwT", [C8 + 1, C8], F16, kind="ExternalInput").ap()
    d_qwT = nc.dram_tensor("qwT", [C8 + 1, C8], F16, kind="ExternalInput").ap()
    d_vwb = nc.dram_tensor("vwb", [C8 + 1, VN], F16, kind="ExternalInput").ap()
    d_outT = nc.dram_tensor("outT", [128, NB, C], F32,
                            kind="ExternalOutput").ap()

    with tile.TileContext(nc) as tc, ExitStack() as ctx:
        const_pool = ctx.enter_context(tc.tile_pool(name="const", bufs=1))
        big_pool = ctx.enter_context(tc.tile_pool(name="big", bufs=1))

        # ---- constants / inputs ----
        # weights ride the gpsimd SWDGE queue; the two HWDGE queues (sync,
        # scalar) are dedicated to the x16 stream from the first descriptor.
        cwT = const_pool.tile([128, 2, C8], F16)
        kwT = const_pool.tile([C8 + 1, C8], F16)
        qwT = const_pool.tile([C8 + 1, C8], F16)
        vwb = const_pool.tile([C8 + 1, VN], F16)
        cb = const_pool.tile([C8, 1], F32)
        warm = const_pool.tile([128, 128], F16)
        for a in range(2):
            nc.gpsimd.dma_start(cwT[:, a, :], d_cwT[a])
        nc.gpsimd.dma_start(kwT[:], d_kwT)
        nc.gpsimd.dma_start(qwT[:], d_qwT)
        nc.gpsimd.dma_start(vwb[:], d_vwb)
        nc.gpsimd.dma_start(cb[:], d_cb)
        nc.vector.memset(warm[:], 0.0)

        # x16: two c-halves [128, HW] fp16 (matmul operand); 4 batched DMAs
        # of [128, 1024] per half, one HWDGE queue per half — few dma_start
        # calls (each costs ~0.7us of serialized queue overhead) but still
        # fine-grained enough that proj can start on the first slice.
        x16 = [big_pool.tile([128, HW], F16, tag=f"x16_{i}", name=f"x16_{i}")
               for i in range(2)]
        for j in range(4):
            sl = bass.ts(j, HW // 4)
            nc.sync.dma_start(x16[0][:, sl], d_x16[0][:, sl])
            nc.scalar.dma_start(x16[1][:, sl], d_x16[1][:, sl])

        # xT: residual input, [128, nb, 256]: query block nb on partitions.
        # Host-packed partition-major (8KB contiguous per descriptor). Not
        # needed until the first epilogue (~60us), so it rides BEHIND the
        # x16 chunks on the same HWDGE queues — each queue generates
        # descriptors in order, so xT never steals DMA bandwidth from the
        # x16 stream that gates proj.
        xT = big_pool.tile([128, NB, C], F32)
        nc.sync.dma_start(xT[:, 0 : NB // 2, :], d_xT[:, 0 : NB // 2, :])
        nc.scalar.dma_start(xT[:, NB // 2 :, :], d_xT[:, NB // 2 :, :])

        proj = big_pool.tile([C8 + 1, HW], F16)   # row 32 = ones
        nc.vector.memset(proj[C8 : C8 + 1, :], 1.0)
        k4 = big_pool.tile([128, HW], F16)        # k replicated on 4 row-groups
        qT4 = big_pool.tile([128, NQ], F16)       # query half, replicated x4
        vt = big_pool.tile([128, N_MCH * VN], BF16)  # vT' chunks [m=128, 257]

        # ---- psum pools ----
        ps_pool = ctx.enter_context(
            tc.tile_pool(name="ps", bufs=2, space="PSUM"))
        po_pool = ctx.enter_context(
            tc.tile_pool(name="po", bufs=2, space="PSUM"))
        att_pool = ctx.enter_context(tc.tile_pool(name="att", bufs=2))
        out_pool = ctx.enter_context(tc.tile_pool(name="outp", bufs=3))

        def pstile(shape, name):
            return ps_pool.tile(shape, F32, tag="ps", name=name)

        def potile(name):
            # full-bank [128, 512] slots: a psum matmul dst must not
            # straddle a 2KB bank boundary (costs ~20% per-MM otherwise)
            return po_pool.tile([128, 512], F32, tag="po", name=name)

        # PE warmup: self-matmuls on the zeroed tile — no DMA dependency, so
        # the PE starts ramping as soon as the DVE memset lands (~5us) and
        # the HAM clock-gate is released before the real work starts.
        pw = pstile([128, 1536], "pw")
        for _ in range(34):
            nc.tensor.matmul(pw[:, 0:128], warm[:], warm[:])

        # proj = conv_w @ x + conv_b  (K = 256 over 2 chunks), bias by ACT
        for s in range(4):
            pp = pstile([C8, 1024], f"pp{s}")
            for jj in range(2):
                sl = bass.ts(jj, 512)
                gsl = bass.ds(s * 1024 + jj * 512, 512)
                nc.tensor.matmul(pp[:, sl], cwT[:, 0, :], x16[0][:, gsl],
                                 start=True, stop=False)
                nc.tensor.matmul(pp[:, sl], cwT[:, 1, :], x16[1][:, gsl],
                                 start=False, stop=True)
            if s % 2 == 0:
                nc.scalar.activation(
                    proj[0:C8, bass.ds(s * 1024, 1024)], pp[:],
                    mybir.ActivationFunctionType.Identity, bias=cb[:])
            else:
                nc.vector.tensor_scalar_add(
                    proj[0:C8, bass.ds(s * 1024, 1024)], pp[:], cb[:])

        # qT4 = q_w' @ proj' (bias via proj ones-row), x4 col-groups
        for h in range(2):
            pq = pstile([128, 1024], f"pq{h}")
            for jj in range(2):
                sl = bass.ts(jj, 512)
                gsl = bass.ds(h * 1024 + jj * 512, 512)
                for g in range(4):
                    nc.tensor.matmul(pq[bass.ts(g, 32), sl], qwT[:],
                                     proj[:, gsl], tile_position=(0, 32 * g))
            nc.vector.tensor_copy(qT4[:, bass.ds(h * 1024, 1024)], pq[:])

        # k4 = k_w' @ proj' on all 4 col-groups (x4 replication)
        for s in range(4):
            pk = pstile([128, 1024], f"pk{s}")
            for jj in range(2):
                sl = bass.ts(jj, 512)
                gsl = bass.ds(s * 1024 + jj * 512, 512)
                for g in range(4):
                    nc.tensor.matmul(pk[bass.ts(g, 32), sl], kwT[:],
                                     proj[:, gsl], tile_position=(0, 32 * g))
            if s % 2 == 0:
                nc.vector.tensor_copy(k4[:, bass.ds(s * 1024, 1024)], pk[:])
            else:
                nc.scalar.copy(k4[:, bass.ds(s * 1024, 1024)], pk[:])

        # ---- attention ----
        n_sup = NQ // NSUP                # 4 super-blocks of 512 queries
        n_blk = NSUP // NBLK              # 4 attnout blocks per super
        # score groups: 3 m-chunks per group (3 psum banks), last group 2
        GRPS = [3] * 10 + [2]             # 11 groups cover 32 m-chunks
        GOFF = [0, 3, 6, 9, 12, 15, 18, 21, 24, 27, 30]
        n_grp = len(GRPS)
        e_sbs = {}

        def alloc_e(ns):
            e_sbs[ns] = att_pool.tile([128, N_MCH * NSUP], BF16, tag="e_sb",
                                      name=f"e_sb_{ns}")

        def emit_score_group(ns, g):
            nsl = bass.ts(ns, NSUP)
            e_sb = e_sbs[ns]
            gch = GRPS[g]
            ps = pstile([128, 1536], f"ps_{ns}_{g}")
            for i in range(gch):
                mi = GOFF[g] + i
                nc.tensor.matmul(
                    ps[:, bass.ts(i, NSUP)],
                    k4[bass.ts(i, 32), bass.ts(mi, MCH)],
                    qT4[bass.ts(i, 32), nsl],
                    tile_position=(32 * i, 0),
                )
            # split each group's exp across BOTH engines: halves the
            # drain latency of the psum slot (the WAR release that gates
            # this slot's next score group)
            cut = (gch + 1) // 2 if g % 2 == 0 else gch // 2
            lo = bass.ds(GOFF[g] * NSUP, cut * NSUP)
            hi = bass.ds((GOFF[g] + cut) * NSUP, (gch - cut) * NSUP)
            if cut:
                nc.scalar.activation(
                    e_sb[:, lo], ps[:, 0 : cut * NSUP],
                    mybir.ActivationFunctionType.Exp)
            if gch - cut:
                nc.vector.tensor_scalar(
                    e_sb[:, hi].bitcast(I16),
                    ps[:, bass.ds(cut * NSUP, (gch - cut) * NSUP)], SCH_MUL,
                    SCH_ADD, mybir.AluOpType.mult, mybir.AluOpType.add)

        def emit_vt_pair(p):
            # two m-chunks per psum tile (wide ps slot), ONE copy op for
            # both: halves the per-op overhead on the fill-critical
            # ACT/DVE queues, and keeps pv out of the po pool entirely
            # (the po accumulators never WAR against the vt build).
            pv = pstile([128, 2, 512], f"pv{p}")
            for i in range(2):
                nc.tensor.matmul(pv[:, i, 0:VN],
                                 proj[:, bass.ts(2 * p + i, MCH)], vwb[:])
            dst = vt[:, bass.ds(2 * p * VN, 2 * VN)].rearrange(
                "q (a v) -> q a v", v=VN)
            if p % 2 == 0:
                nc.vector.tensor_copy(dst, pv[:, :, 0:VN])
            else:
                nc.scalar.copy(dst, pv[:, :, 0:VN])

        def emit_block_epilogue(po, nbg):
            rcol = out_pool.tile([128, 1], F32, tag="rcol",
                                 name=f"rcol_{nbg}")
            nc.vector.reciprocal(rcol[:], po[:, C : C + 1])
            # fused (po * rcol) + xT in one DVE pass
            osb = out_pool.tile([128, C], F32, tag="osb", name=f"osb_{nbg}")
            nc.vector.scalar_tensor_tensor(
                osb[:], po[:, 0:C], rcol[:], xT[:, nbg, :],
                mybir.AluOpType.mult, mybir.AluOpType.add)
            nc.sync.dma_start(d_outT[:, nbg, :], osb[:])

        def emit_attnout_half(po, e_sb, nb, half):
            for mi in range(half * 16, half * 16 + 16):
                nc.tensor.matmul(
                    po[:, 0:VN],
                    e_sb[:, bass.ds(mi * NSUP + nb * NBLK, NBLK)],
                    vt[:, bass.ts(mi, VN)],
                    start=(mi == 0), stop=(mi == N_MCH - 1),
                )

        def emit_attnout_pair(ns, nbs):
            # interleave two blocks' accumulation chains chunk-by-chunk
            e_sb = e_sbs[ns]
            pos = [potile(f"pot_{ns}_{nb}") for nb in nbs]
            for mi in range(N_MCH):
                for po, nb in zip(pos, nbs):
                    nc.tensor.matmul(
                        po[:, 0:VN],
                        e_sb[:, bass.ds(mi * NSUP + nb * NBLK, NBLK)],
                        vt[:, bass.ts(mi, VN)],
                        start=(mi == 0), stop=(mi == N_MCH - 1),
                    )
            for po, nb in zip(pos, nbs):
                emit_block_epilogue(po, ns * n_blk + nb)

        # super-0 fill: score groups interleaved with vt pairs; both
        # rotate through the ps pool. The fill is ACT/DVE-bound (exp
        # stream + vt copies), so PE-side WAR stalls here are free.
        alloc_e(0)
        for g in range(n_grp):
            emit_score_group(0, g)
            emit_vt_pair(g)
        for p in range(n_grp, N_MCH // 2):
            emit_vt_pair(p)
        # scheduler fence: the attention steady-state must NOT overlap the
        # fill tail — co-running attnout blocks with the fill's exp/copy
        # stream slows every PE matmul ~15% (SBUF/weight-path contention)
        # and stretches the attention phase far more than the overlap wins.
        tc.no_sync_barrier()

        # steady state: per attnout block of super S, ~3 score groups of
        # super S+1 are emitted BETWEEN the two halves of the block's
        # matmul chain, so each group's exp (ACT or DVE, ~1.6-1.9us) has a
        # half-block (~1.8us) of PE work to drain before its psum slot
        # comes around again. The final super has no successor scores; its
        # blocks run as interleaved pairs.
        # attnout as pair-interleaved block chains (two psum accumulators
        # in flight hide the per-MM accumulate latency), with the next
        # super's score groups bursted between pair-halves.
        GSLOT = [[0, 1, 2], [3, 4, 5], [6, 7, 8], [9, 10]]
        for ns in range(n_sup):
            nxt = ns + 1 < n_sup
            if nxt:
                alloc_e(ns + 1)
            e_sb = e_sbs[ns]
            for nbp in (0, 2):
                nbs = [nbp, nbp + 1]
                pos = [potile(f"po_{ns}_{nb}") for nb in nbs]
                for half in (0, 1):
                    for mi in range(half * 16, half * 16 + 16):
                        for po, nb in zip(pos, nbs):
                            nc.tensor.matmul(
                                po[:, 0:VN],
                                e_sb[:, bass.ds(mi * NSUP + nb * NBLK, NBLK)],
                                vt[:, bass.ts(mi, VN)],
                                start=(mi == 0), stop=(mi == N_MCH - 1),
                            )
                    if nxt:
                        for g in GSLOT[nbp + half]:
                            emit_score_group(ns + 1, g)
                for po, nb in zip(pos, nbs):
                    emit_block_epilogue(po, ns * n_blk + nb)
            e_sbs.pop(ns)

    nc.compile()
    return nc


def _prep_in_maps(x, conv_w, conv_b, q_w, q_b, k_w, k_b, v_w, v_b, gamma):
    g = np.float32(gamma[0])
    cwT = np.ascontiguousarray(conv_w.T.reshape(2, 128, C8)).astype(np.float16)
    kwT = np.concatenate([k_w.T, k_b[None, :]], axis=0).astype(np.float16)
    qwT = np.concatenate([q_w.T, q_b[None, :]], axis=0).astype(np.float16)
    vwb = np.zeros((C8 + 1, VN), np.float16)
    vwb[0:C8, 0:C] = (g * v_w).T.astype(np.float16)
    vwb[C8, 0:C] = (g * v_b).astype(np.float16)
    vwb[C8, C] = 1.0
    cb = conv_b.reshape(C8, 1).astype(np.float32)

    in_maps = []
    for core in range(8):
        b, hf = core // 2, core % 2
        xf = np.asarray(x[b], np.float32).reshape(C, HW)
        if hf:
            # rotate spatial columns: this core's query half -> cols 0:2048
            xf = np.roll(xf, -NQ, axis=1)
        # xT packed partition-major: [128 p, 16 nb, 256 c]
        xTp = np.ascontiguousarray(
            xf[:, 0:NQ].T.reshape(NB, 128, C).transpose(1, 0, 2))
        in_maps.append({
            "x16": np.ascontiguousarray(xf.reshape(2, 128, HW)).astype(
                np.float16),
            "xT": xTp,
            "cwT": cwT, "cb": cb, "kwT": kwT, "qwT": qwT, "vwb": vwb,
        })
    return in_maps


def kernel(x, conv_w, conv_b, q_w, q_b, k_w, k_b, v_w, v_b, gamma, **run_kw):
    if "nc" not in _CACHED:
        _CACHED["nc"] = build_nc()
    nc = _CACHED["nc"]
    in_maps = _prep_in_maps(x, conv_w, conv_b, q_w, q_b, k_w, k_b, v_w, v_b,
                            gamma)
    res = run_bass_kernel_spmd(nc, in_maps, core_ids=list(range(8)), **run_kw)
    _CACHED["last_result"] = res
    out = np.empty((B, C, HW), np.float32)
    for core in range(8):
        b, hf = core // 2, core % 2
        oc = np.asarray(res.results[core]["outT"])  # [128, 16, 256]
        out[b, :, hf * NQ : (hf + 1) * NQ] = \
            oc.transpose(1, 0, 2).reshape(NQ, C).T
    return out.reshape(B, C, H, W)


# revision 21
# speedup vs baseline: 1.0567x; 1.0567x over previous
"""TRN2 Bass kernel for nn_AttentionModule (dense transformer attention block).

Reference computation (per sample b, x flattened to [256, 4096]):
    proj = conv_w @ x + conv_b                 [32, 4096]
    q    = (q_w @ proj + q_b).T                [4096, 32]
    k    = k_w @ proj + k_b                    [32, 4096]
    v    = v_w @ proj + v_b                    [256, 4096]
    attn = softmax(q @ k, axis=-1)             [4096(n), 4096(m)]
    out  = gamma * (v @ attn.T) + x            [256, 4096]

Sharding: 8 cores = 4 samples x 2 query-halves (2048 queries each). Each core
redundantly computes proj/k/v for its sample (cheap) and its half of the
queries. No cross-core communication. SPMD: odd cores receive x with the
spatial axis rolled by -2048 so "their" queries sit at columns 0:2048;
attention is permutation-invariant over keys so k/v column order is free.

On-core layout: scores are computed transposed, [m_keys(part), n_queries
(free)], so the exp'd scores chunks are directly usable as matmul weights
(lhsT) for the attn@V contraction over m, and the softmax denominator falls
out of the same matmul via an appended ones-column in the V^T projection
(column 256 of the [33,257] rhs; proj carries a ones-row 32 that also folds
in the v bias). No max-subtraction: exp'd scores are stored in bf16 (no
overflow below e^88); numerator and denominator share the same bf16 rounding
so softmax normalization cancels most of it. The residual is applied in
[n, c] layout against a host-transposed x, and the host transposes the
[128, 16, 256] per-core output back — no on-chip transposes at all.

PSUM layout: two pools. ps_pool = 2 slots x 3 banks [128, 1536] for score
groups (3 m-chunks each); po_pool = 2 slots x 1 bank [128, 512] for the
attnout accumulators and the vT' build. This keeps the attnout accumulation
chain OFF the score-slot rotation: a po allocation never has to wait for an
exp to drain its psum (which used to put the ~2us exp latency on the PE
critical path once per block).

The exp stream is split between ACT (true Exp) and DVE (Schraudolph bit-trick
exp: int16(round(s*128/ln2 + 127*128 - 5.59)) bitcast as bf16, ~3% relative
error that largely cancels between softmax num/denom). Score groups are
emitted between HALF attnout blocks so a group's exp has ~2 block-halves of
PE work to drain before its psum slot is reused.

gamma is folded into v_w/v_b host-side. fp16 feeds the q/k score path.
"""

import numpy as np
from contextlib import ExitStack

import concourse.bass as bass
import concourse.bacc as bacc
import concourse.tile as tile
from concourse import mybir
from concourse.bass_utils import run_bass_kernel_spmd

F32 = mybir.dt.float32
F16 = mybir.dt.float16
BF16 = mybir.dt.bfloat16
I16 = mybir.dt.int16

B, C, H, W = 4, 256, 64, 64
HW = H * W          # 4096 keys (m)
NQ = HW // 2        # 2048 queries per core (n)
C8 = 32             # qk head dim (e) / proj channels (d)
NSUP = 512          # queries per attention super-block
NBLK = 128          # queries per attnout block
MCH = 128           # keys per m-chunk (one lhsT tile)
N_MCH = HW // MCH   # 32 m-chunks
VN = C + 1          # 257: v channels + ones column (softmax denominator)
NB = NQ // NBLK     # 16 attnout blocks

# Schraudolph fast-exp constants (bf16 bit pattern via int16 affine)
SCH_MUL = 184.6650292
SCH_ADD = 16250.41

_CACHED = {}


def build_nc():
    nc = bacc.Bacc("TRN2", target_bir_lowering=False, debug=False)
    d_x16 = nc.dram_tensor("x16", [2, 128, HW], F16, kind="ExternalInput").ap()
    d_xT = nc.dram_tensor("xT", [128, NB, C], F32, kind="ExternalInput").ap()
    d_cwT = nc.dram_tensor("cwT", [2, 128, C8], F16, kind="ExternalInput").ap()
    d_cb = nc.dram_tensor("cb", [C8, 1], F32, kind="ExternalInput").ap()
    # k/q weights carry their bias as row 32, contracted against proj's
    # ones-row — no separate bias op needed.
    d_kwT = nc.dram_tensor("kwT", [C8 + 1, C8], F16, kind="ExternalInput").ap()
    d_qwT = nc.dram_tensor("qwT", [C8 + 1, C8], F16, kind="ExternalInput").ap()
    d_vwb = nc.dram_tensor("vwb", [C8 + 1, VN], F16, kind="ExternalInput").ap()
    d_outT = nc.dram_tensor("outT", [128, NB, C], F32,
                            kind="ExternalOutput").ap()

    with tile.TileContext(nc) as tc, ExitStack() as ctx:
        const_pool = ctx.enter_context(tc.tile_pool(name="const", bufs=1))
        big_pool = ctx.enter_context(tc.tile_pool(name="big", bufs=1))

        # ---- constants / inputs ----
        # weights ride the gpsimd SWDGE queue; the two HWDGE queues (sync,
        # scalar) are dedicated to the x16 stream from the first descriptor.
        cwT = const_pool.tile([128, 2, C8], F16)
        kwT = const_pool.tile([C8 + 1, C8], F16)
        qwT = const_pool.tile([C8 + 1, C8], F16)
        vwb = const_pool.tile([C8 + 1, VN], F16)
        cb = const_pool.tile([C8, 1], F32)
        warm = const_pool.tile([128, 128], F16)
        for a in range(2):
            nc.gpsimd.dma_start(cwT[:, a, :], d_cwT[a])
        nc.gpsimd.dma_start(kwT[:], d_kwT)
        nc.gpsimd.dma_start(qwT[:], d_qwT)
        nc.gpsimd.dma_start(vwb[:], d_vwb)
        nc.gpsimd.dma_start(cb[:], d_cb)
        nc.vector.memset(warm[:], 0.0)

        # x16: two c-halves [128, HW] fp16 (matmul operand); 4 batched DMAs
        # of [128, 1024] per half, one HWDGE queue per half — few dma_start
        # calls (each costs ~0.7us of serialized queue overhead) but still
        # fine-grained enough that proj can start on the first slice.
        x16 = [big_pool.tile([128, HW], F16, tag=f"x16_{i}", name=f"x16_{i}")
               for i in range(2)]
        for j in range(4):
            sl = bass.ts(j, HW // 4)
            nc.sync.dma_start(x16[0][:, sl], d_x16[0][:, sl])
            nc.scalar.dma_start(x16[1][:, sl], d_x16[1][:, sl])

        # xT: residual input, [128, nb, 256]: query block nb on partitions.
        # Host-packed partition-major (8KB contiguous per descriptor). Not
        # needed until the first epilogue (~60us), so it rides BEHIND the
        # x16 chunks on the same HWDGE queues — each queue generates
        # descriptors in order, so xT never steals DMA bandwidth from the
        # x16 stream that gates proj.
        xT = big_pool.tile([128, NB, C], F32)
        nc.sync.dma_start(xT[:, 0 : NB // 2, :], d_xT[:, 0 : NB // 2, :])
        nc.scalar.dma_start(xT[:, NB // 2 :, :], d_xT[:, NB // 2 :, :])

        proj = big_pool.tile([C8 + 1, HW], F16)   # row 32 = ones
        nc.vector.memset(proj[C8 : C8 + 1, :], 1.0)
        k4 = big_pool.tile([128, HW], F16)        # k replicated on 4 row-groups
        qT4 = big_pool.tile([128, NQ], F16)       # query half, replicated x4
        vt = big_pool.tile([128, N_MCH * VN], BF16)  # vT' chunks [m=128, 257]

        # ---- psum pools ----
        ps_pool = ctx.enter_context(
            tc.tile_pool(name="ps", bufs=2, space="PSUM"))
        po_pool = ctx.enter_context(
            tc.tile_pool(name="po", bufs=2, space="PSUM"))
        att_pool = ctx.enter_context(tc.tile_pool(name="att", bufs=2))
        out_pool = ctx.enter_context(tc.tile_pool(name="outp", bufs=3))

        def pstile(shape, name):
            return ps_pool.tile(shape, F32, tag="ps", name=name)

        def potile(name):
            # full-bank [128, 512] slots: a psum matmul dst must not
            # straddle a 2KB bank boundary (costs ~20% per-MM otherwise)
            return po_pool.tile([128, 512], F32, tag="po", name=name)

        # PE warmup: self-matmuls on the zeroed tile — no DMA dependency, so
        # the PE starts ramping as soon as the DVE memset lands (~5us) and
        # the HAM clock-gate is released before the real work starts.
        pw = pstile([128, 1536], "pw")
        for _ in range(34):
            nc.tensor.matmul(pw[:, 0:128], warm[:], warm[:])

        # proj = conv_w @ x + conv_b  (K = 256 over 2 chunks), bias by ACT
        for s in range(4):
            pp = pstile([C8, 1024], f"pp{s}")
            for jj in range(2):
                sl = bass.ts(jj, 512)
                gsl = bass.ds(s * 1024 + jj * 512, 512)
                nc.tensor.matmul(pp[:, sl], cwT[:, 0, :], x16[0][:, gsl],
                                 start=True, stop=False)
                nc.tensor.matmul(pp[:, sl], cwT[:, 1, :], x16[1][:, gsl],
                                 start=False, stop=True)
            nc.scalar.activation(
                proj[0:C8, bass.ds(s * 1024, 1024)], pp[:],
                mybir.ActivationFunctionType.Identity, bias=cb[:])

        # qT4 = q_w' @ proj' (bias via proj ones-row), x4 col-groups
        for h in range(2):
            pq = pstile([128, 1024], f"pq{h}")
            for jj in range(2):
                sl = bass.ts(jj, 512)
                gsl = bass.ds(h * 1024 + jj * 512, 512)
                for g in range(4):
                    nc.tensor.matmul(pq[bass.ts(g, 32), sl], qwT[:],
                                     proj[:, gsl], tile_position=(0, 32 * g))
            nc.vector.tensor_copy(qT4[:, bass.ds(h * 1024, 1024)], pq[:])

        # k4 = k_w' @ proj' on all 4 col-groups (x4 replication)
        for s in range(4):
            pk = pstile([128, 1024], f"pk{s}")
            for jj in range(2):
                sl = bass.ts(jj, 512)
                gsl = bass.ds(s * 1024 + jj * 512, 512)
                for g in range(4):
                    nc.tensor.matmul(pk[bass.ts(g, 32), sl], kwT[:],
                                     proj[:, gsl], tile_position=(0, 32 * g))
            if s % 2 == 0:
                nc.vector.tensor_copy(k4[:, bass.ds(s * 1024, 1024)], pk[:])
            else:
                nc.scalar.copy(k4[:, bass.ds(s * 1024, 1024)], pk[:])

        # ---- attention ----
        n_sup = NQ // NSUP                # 4 super-blocks of 512 queries
        n_blk = NSUP // NBLK              # 4 attnout blocks per super
        # score groups: 3 m-chunks per group (3 psum banks), last group 2
        GRPS = [3] * 10 + [2]             # 11 groups cover 32 m-chunks
        GOFF = [0, 3, 6, 9, 12, 15, 18, 21, 24, 27, 30]
        n_grp = len(GRPS)
        e_sbs = {}

        def alloc_e(ns):
            e_sbs[ns] = att_pool.tile([128, N_MCH * NSUP], BF16, tag="e_sb",
                                      name=f"e_sb_{ns}")

        def emit_score_group(ns, g):
            nsl = bass.ts(ns, NSUP)
            e_sb = e_sbs[ns]
            gch = GRPS[g]
            ps = pstile([128, 1536], f"ps_{ns}_{g}")
            for i in range(gch):
                mi = GOFF[g] + i
                nc.tensor.matmul(
                    ps[:, bass.ts(i, NSUP)],
                    k4[bass.ts(i, 32), bass.ts(mi, MCH)],
                    qT4[bass.ts(i, 32), nsl],
                    tile_position=(32 * i, 0),
                )
            # split each group's exp across BOTH engines: halves the
            # drain latency of the psum slot (the WAR release that gates
            # this slot's next score group)
            cut = (gch + 1) // 2 if g % 2 == 0 else gch // 2
            lo = bass.ds(GOFF[g] * NSUP, cut * NSUP)
            hi = bass.ds((GOFF[g] + cut) * NSUP, (gch - cut) * NSUP)
            if cut:
                nc.scalar.activation(
                    e_sb[:, lo], ps[:, 0 : cut * NSUP],
                    mybir.ActivationFunctionType.Exp)
            if gch - cut:
                nc.vector.tensor_scalar(
                    e_sb[:, hi].bitcast(I16),
                    ps[:, bass.ds(cut * NSUP, (gch - cut) * NSUP)], SCH_MUL,
                    SCH_ADD, mybir.AluOpType.mult, mybir.AluOpType.add)

        def emit_vt_chunk(mi):
            pv = potile(f"pv{mi}")
            nc.tensor.matmul(pv[:, 0:VN], proj[:, bass.ts(mi, MCH)], vwb[:])
            if mi % 2 == 0:
                nc.vector.tensor_copy(vt[:, bass.ts(mi, VN)], pv[:, 0:VN])
            else:
                nc.scalar.copy(vt[:, bass.ts(mi, VN)], pv[:, 0:VN])

        def emit_block_epilogue(po, nbg):
            rcol = out_pool.tile([128, 1], F32, tag="rcol",
                                 name=f"rcol_{nbg}")
            nc.vector.reciprocal(rcol[:], po[:, C : C + 1])
            # fused (po * rcol) + xT in one DVE pass
            osb = out_pool.tile([128, C], F32, tag="osb", name=f"osb_{nbg}")
            nc.vector.scalar_tensor_tensor(
                osb[:], po[:, 0:C], rcol[:], xT[:, nbg, :],
                mybir.AluOpType.mult, mybir.AluOpType.add)
            nc.sync.dma_start(d_outT[:, nbg, :], osb[:])

        def emit_attnout_half(po, e_sb, nb, half):
            for mi in range(half * 16, half * 16 + 16):
                nc.tensor.matmul(
                    po[:, 0:VN],
                    e_sb[:, bass.ds(mi * NSUP + nb * NBLK, NBLK)],
                    vt[:, bass.ts(mi, VN)],
                    start=(mi == 0), stop=(mi == N_MCH - 1),
                )

        def emit_attnout_pair(ns, nbs):
            # interleave two blocks' accumulation chains chunk-by-chunk
            e_sb = e_sbs[ns]
            pos = [potile(f"pot_{ns}_{nb}") for nb in nbs]
            for mi in range(N_MCH):
                for po, nb in zip(pos, nbs):
                    nc.tensor.matmul(
                        po[:, 0:VN],
                        e_sb[:, bass.ds(mi * NSUP + nb * NBLK, NBLK)],
                        vt[:, bass.ts(mi, VN)],
                        start=(mi == 0), stop=(mi == N_MCH - 1),
                    )
            for po, nb in zip(pos, nbs):
                emit_block_epilogue(po, ns * n_blk + nb)

        # super 0: scores + exp interleaved with the vT' build. vt chunk
        # matmuls ride between score groups; copies alternate DVE/ACT. The
        # vt chunks share the po pool, which conveniently throttles the
        # first attnout block until the vT' build has drained.
        alloc_e(0)
        vt_done = 0
        for g in range(n_grp):
            emit_score_group(0, g)
            n_vt = 3 if g < 10 else 2
            for _ in range(n_vt):
                if vt_done < N_MCH:
                    emit_vt_chunk(vt_done)
                    vt_done += 1
        while vt_done < N_MCH:
            emit_vt_chunk(vt_done)
            vt_done += 1

        # steady state: per attnout block of super S, ~3 score groups of
        # super S+1 are emitted BETWEEN the two halves of the block's
        # matmul chain, so each group's exp (ACT or DVE, ~1.6-1.9us) has a
        # half-block (~1.8us) of PE work to drain before its psum slot
        # comes around again. The final super has no successor scores; its
        # blocks run as interleaved pairs.
        GSLOT = [[0, 1, 2], [3, 4, 5], [6, 7, 8], [9, 10]]
        for ns in range(n_sup):
            if ns + 1 < n_sup:
                alloc_e(ns + 1)
                for nb in range(n_blk):
                    e_sb = e_sbs[ns]
                    po = potile(f"po_{ns}_{nb}")
                    emit_attnout_half(po, e_sb, nb, 0)
                    emit_attnout_half(po, e_sb, nb, 1)
                    for g in GSLOT[nb]:
                        emit_score_group(ns + 1, g)
                    emit_block_epilogue(po, ns * n_blk + nb)
            else:
                emit_attnout_pair(ns, [0, 1])
                emit_attnout_pair(ns, [2, 3])
            e_sbs.pop(ns)

    nc.compile()
    return nc


def _prep_in_maps(x, conv_w, conv_b, q_w, q_b, k_w, k_b, v_w, v_b, gamma):
    g = np.float32(gamma[0])
    cwT = np.ascontiguousarray(conv_w.T.reshape(2, 128, C8)).astype(np.float16)
    kwT = np.concatenate([k_w.T, k_b[None, :]], axis=0).astype(np.float16)
    qwT = np.concatenate([q_w.T, q_b[None, :]], axis=0).astype(np.float16)
    vwb = np.zeros((C8 + 1, VN), np.float16)
    vwb[0:C8, 0:C] = (g * v_w).T.astype(np.float16)
    vwb[C8, 0:C] = (g * v_b).astype(np.float16)
    vwb[C8, C] = 1.0
    cb = conv_b.reshape(C8, 1).astype(np.float32)

    in_maps = []
    for core in range(8):
        b, hf = core // 2, core % 2
        xf = np.asarray(x[b], np.float32).reshape(C, HW)
        if hf:
            # rotate spatial columns: this core's query half -> cols 0:2048
            xf = np.roll(xf, -NQ, axis=1)
        # xT packed partition-major: [128 p, 16 nb, 256 c]
        xTp = np.ascontiguousarray(
            xf[:, 0:NQ].T.reshape(NB, 128, C).transpose(1, 0, 2))
        in_maps.append({
            "x16": np.ascontiguousarray(xf.reshape(2, 128, HW)).astype(
                np.float16),
            "xT": xTp,
            "cwT": cwT, "cb": cb, "kwT": kwT, "qwT": qwT, "vwb": vwb,
        })
    return in_maps


def kernel(x, conv_w, conv_b, q_w, q_b, k_w, k_b, v_w, v_b, gamma, **run_kw):
    if "nc" not in _CACHED:
        _CACHED["nc"] = build_nc()
    nc = _CACHED["nc"]
    in_maps = _prep_in_maps(x, conv_w, conv_b, q_w, q_b, k_w, k_b, v_w, v_b,
                            gamma)
    res = run_bass_kernel_spmd(nc, in_maps, core_ids=list(range(8)), **run_kw)
    _CACHED["last_result"] = res
    out = np.empty((B, C, HW), np.float32)
    for core in range(8):
        b, hf = core // 2, core % 2
        oc = np.asarray(res.results[core]["outT"])  # [128, 16, 256]
        out[b, :, hf * NQ : (hf + 1) * NQ] = \
            oc.transpose(1, 0, 2).reshape(NQ, C).T
    return out.reshape(B, C, H, W)


# revision 22
# speedup vs baseline: 1.0609x; 1.0040x over previous
"""TRN2 Bass kernel for nn_AttentionModule (dense transformer attention block).

Reference computation (per sample b, x flattened to [256, 4096]):
    proj = conv_w @ x + conv_b                 [32, 4096]
    q    = (q_w @ proj + q_b).T                [4096, 32]
    k    = k_w @ proj + k_b                    [32, 4096]
    v    = v_w @ proj + v_b                    [256, 4096]
    attn = softmax(q @ k, axis=-1)             [4096(n), 4096(m)]
    out  = gamma * (v @ attn.T) + x            [256, 4096]

Sharding: 8 cores = 4 samples x 2 query-halves (2048 queries each). Each core
redundantly computes proj/k/v for its sample (cheap) and its half of the
queries. No cross-core communication. SPMD: odd cores receive x with the
spatial axis rolled by -2048 so "their" queries sit at columns 0:2048;
attention is permutation-invariant over keys so k/v column order is free.

On-core layout: scores are computed transposed, [m_keys(part), n_queries
(free)], so the exp'd scores chunks are directly usable as matmul weights
(lhsT) for the attn@V contraction over m, and the softmax denominator falls
out of the same matmul via an appended ones-column in the V^T projection
(column 256 of the [33,257] rhs; proj carries a ones-row 32 that also folds
in the v bias). No max-subtraction: exp'd scores are stored in bf16 (no
overflow below e^88); numerator and denominator share the same bf16 rounding
so softmax normalization cancels most of it. The residual is applied in
[n, c] layout against a host-transposed x, and the host transposes the
[128, 16, 256] per-core output back — no on-chip transposes at all.

PSUM layout: two pools. ps_pool = 2 slots x 3 banks [128, 1536] for score
groups (3 m-chunks each); po_pool = 2 slots x 1 bank [128, 512] for the
attnout accumulators and the vT' build. This keeps the attnout accumulation
chain OFF the score-slot rotation: a po allocation never has to wait for an
exp to drain its psum (which used to put the ~2us exp latency on the PE
critical path once per block).

The exp stream is split between ACT (true Exp) and DVE (Schraudolph bit-trick
exp: int16(round(s*128/ln2 + 127*128 - 5.59)) bitcast as bf16, ~3% relative
error that largely cancels between softmax num/denom); each group's exp is
further split across BOTH engines to halve its drain latency. Score groups
for super S+1 are emitted as bursts after each attnout block of super S, so
a group's exp has a full block (~4us) of PE work to drain before its psum
slot is reused. In the fill phase the vT' build shares the po pool, which
throttles the first attnout block until the build has drained — overlapping
the attention steady-state with the fill tail measurably slows every PE
matmul (weight-path/SBUF contention), so the phases are kept separate.

gamma is folded into v_w/v_b host-side. fp16 feeds the q/k score path.
"""

import numpy as np
from contextlib import ExitStack

import concourse.bass as bass
import concourse.bacc as bacc
import concourse.tile as tile
from concourse import mybir
from concourse.bass_utils import run_bass_kernel_spmd

F32 = mybir.dt.float32
F16 = mybir.dt.float16
BF16 = mybir.dt.bfloat16
I16 = mybir.dt.int16

B, C, H, W = 4, 256, 64, 64
HW = H * W          # 4096 keys (m)
NQ = HW // 2        # 2048 queries per core (n)
C8 = 32             # qk head dim (e) / proj channels (d)
NSUP = 512          # queries per attention super-block
NBLK = 128          # queries per attnout block
MCH = 128           # keys per m-chunk (one lhsT tile)
N_MCH = HW // MCH   # 32 m-chunks
VN = C + 1          # 257: v channels + ones column (softmax denominator)
NB = NQ // NBLK     # 16 attnout blocks

# Schraudolph fast-exp constants (bf16 bit pattern via int16 affine)
SCH_MUL = 184.6650292
SCH_ADD = 16250.41

_CACHED = {}


def build_nc():
    nc = bacc.Bacc("TRN2", target_bir_lowering=False, debug=False)
    d_x16 = nc.dram_tensor("x16", [2, 128, HW], F16, kind="ExternalInput").ap()
    d_xT = nc.dram_tensor("xT", [128, NB, C], F32, kind="ExternalInput").ap()
    d_cwT = nc.dram_tensor("cwT", [2, 128, C8], F16, kind="ExternalInput").ap()
    d_cb = nc.dram_tensor("cb", [C8, 1], F32, kind="ExternalInput").ap()
    # k/q weights carry their bias as row 32, contracted against proj's
    # ones-row — no separate bias op needed.
    d_kwT = nc.dram_tensor("kwT", [C8 + 1, C8], F16, kind="ExternalInput").ap()
    d_qwT = nc.dram_tensor("qwT", [C8 + 1, C8], F16, kind="ExternalInput").ap()
    d_vwb = nc.dram_tensor("vwb", [C8 + 1, VN], F16, kind="ExternalInput").ap()
    d_outT = nc.dram_tensor("outT", [128, NB, C], F32,
                            kind="ExternalOutput").ap()

    with tile.TileContext(nc) as tc, ExitStack() as ctx:
        const_pool = ctx.enter_context(tc.tile_pool(name="const", bufs=1))
        big_pool = ctx.enter_context(tc.tile_pool(name="big", bufs=1))

        # ---- constants / inputs ----
        # weights ride the gpsimd SWDGE queue; the two HWDGE queues (sync,
        # scalar) are dedicated to the x16 stream from the first descriptor.
        cwT = const_pool.tile([128, 2, C8], F16)
        kwT = const_pool.tile([C8 + 1, C8], F16)
        qwT = const_pool.tile([C8 + 1, C8], F16)
        vwb = const_pool.tile([C8 + 1, VN], F16)
        cb = const_pool.tile([C8, 1], F32)
        warm = const_pool.tile([128, 128], F16)
        for a in range(2):
            nc.gpsimd.dma_start(cwT[:, a, :], d_cwT[a])
        nc.gpsimd.dma_start(kwT[:], d_kwT)
        nc.gpsimd.dma_start(qwT[:], d_qwT)
        nc.gpsimd.dma_start(vwb[:], d_vwb)
        nc.gpsimd.dma_start(cb[:], d_cb)
        nc.vector.memset(warm[:], 0.0)

        # x16: two c-halves [128, HW] fp16 (matmul operand); 4 batched DMAs
        # of [128, 1024] per half, one HWDGE queue per half — few dma_start
        # calls (each costs ~0.7us of serialized queue overhead) but still
        # fine-grained enough that proj can start on the first slice.
        x16 = [big_pool.tile([128, HW], F16, tag=f"x16_{i}", name=f"x16_{i}")
               for i in range(2)]
        for j in range(4):
            sl = bass.ts(j, HW // 4)
            nc.sync.dma_start(x16[0][:, sl], d_x16[0][:, sl])
            nc.scalar.dma_start(x16[1][:, sl], d_x16[1][:, sl])

        # xT: residual input, [128, nb, 256]: query block nb on partitions.
        # Host-packed partition-major (8KB contiguous per descriptor). Not
        # needed until the first epilogue (~60us), so it rides BEHIND the
        # x16 chunks on the same HWDGE queues — each queue generates
        # descriptors in order, so xT never steals DMA bandwidth from the
        # x16 stream that gates proj.
        xT = big_pool.tile([128, NB, C], F32)
        nc.sync.dma_start(xT[:, 0 : NB // 2, :], d_xT[:, 0 : NB // 2, :])
        nc.scalar.dma_start(xT[:, NB // 2 :, :], d_xT[:, NB // 2 :, :])

        proj = big_pool.tile([C8 + 1, HW], F16)   # row 32 = ones
        nc.vector.memset(proj[C8 : C8 + 1, :], 1.0)
        k4 = big_pool.tile([128, HW], F16)        # k replicated on 4 row-groups
        qT4 = big_pool.tile([128, NQ], F16)       # query half, replicated x4
        vt = big_pool.tile([128, N_MCH * VN], BF16)  # vT' chunks [m=128, 257]

        # ---- psum pools ----
        ps_pool = ctx.enter_context(
            tc.tile_pool(name="ps", bufs=2, space="PSUM"))
        po_pool = ctx.enter_context(
            tc.tile_pool(name="po", bufs=2, space="PSUM"))
        att_pool = ctx.enter_context(tc.tile_pool(name="att", bufs=2))
        out_pool = ctx.enter_context(tc.tile_pool(name="outp", bufs=3))

        def pstile(shape, name):
            return ps_pool.tile(shape, F32, tag="ps", name=name)

        def potile(name):
            # full-bank [128, 512] slots: a psum matmul dst must not
            # straddle a 2KB bank boundary (costs ~20% per-MM otherwise)
            return po_pool.tile([128, 512], F32, tag="po", name=name)

        # PE warmup: self-matmuls on the zeroed tile — no DMA dependency, so
        # the PE starts ramping as soon as the DVE memset lands (~5us) and
        # the HAM clock-gate is released before the real work starts.
        pw = pstile([128, 1536], "pw")
        for _ in range(34):
            nc.tensor.matmul(pw[:, 0:128], warm[:], warm[:])

        # proj = conv_w @ x + conv_b  (K = 256 over 2 chunks), bias by ACT
        for s in range(4):
            pp = pstile([C8, 1024], f"pp{s}")
            for jj in range(2):
                sl = bass.ts(jj, 512)
                gsl = bass.ds(s * 1024 + jj * 512, 512)
                nc.tensor.matmul(pp[:, sl], cwT[:, 0, :], x16[0][:, gsl],
                                 start=True, stop=False)
                nc.tensor.matmul(pp[:, sl], cwT[:, 1, :], x16[1][:, gsl],
                                 start=False, stop=True)
            nc.scalar.activation(
                proj[0:C8, bass.ds(s * 1024, 1024)], pp[:],
                mybir.ActivationFunctionType.Identity, bias=cb[:])

        # qT4 = q_w' @ proj' (bias via proj ones-row), x4 col-groups
        for h in range(2):
            pq = pstile([128, 1024], f"pq{h}")
            for jj in range(2):
                sl = bass.ts(jj, 512)
                gsl = bass.ds(h * 1024 + jj * 512, 512)
                for g in range(4):
                    nc.tensor.matmul(pq[bass.ts(g, 32), sl], qwT[:],
                                     proj[:, gsl], tile_position=(0, 32 * g))
            nc.vector.tensor_copy(qT4[:, bass.ds(h * 1024, 1024)], pq[:])

        # k4 = k_w' @ proj' on all 4 col-groups (x4 replication)
        for s in range(4):
            pk = pstile([128, 1024], f"pk{s}")
            for jj in range(2):
                sl = bass.ts(jj, 512)
                gsl = bass.ds(s * 1024 + jj * 512, 512)
                for g in range(4):
                    nc.tensor.matmul(pk[bass.ts(g, 32), sl], kwT[:],
                                     proj[:, gsl], tile_position=(0, 32 * g))
            if s % 2 == 0:
                nc.vector.tensor_copy(k4[:, bass.ds(s * 1024, 1024)], pk[:])
            else:
                nc.scalar.copy(k4[:, bass.ds(s * 1024, 1024)], pk[:])

        # ---- attention ----
        n_sup = NQ // NSUP                # 4 super-blocks of 512 queries
        n_blk = NSUP // NBLK              # 4 attnout blocks per super
        # score groups: 3 m-chunks per group (3 psum banks), last group 2
        GRPS = [3] * 10 + [2]             # 11 groups cover 32 m-chunks
        GOFF = [0, 3, 6, 9, 12, 15, 18, 21, 24, 27, 30]
        n_grp = len(GRPS)
        e_sbs = {}

        def alloc_e(ns):
            e_sbs[ns] = att_pool.tile([128, N_MCH * NSUP], BF16, tag="e_sb",
                                      name=f"e_sb_{ns}")

        def emit_score_group(ns, g):
            nsl = bass.ts(ns, NSUP)
            e_sb = e_sbs[ns]
            gch = GRPS[g]
            ps = pstile([128, 1536], f"ps_{ns}_{g}")
            for i in range(gch):
                mi = GOFF[g] + i
                nc.tensor.matmul(
                    ps[:, bass.ts(i, NSUP)],
                    k4[bass.ts(i, 32), bass.ts(mi, MCH)],
                    qT4[bass.ts(i, 32), nsl],
                    tile_position=(32 * i, 0),
                )
            # split each group's exp across BOTH engines: halves the
            # drain latency of the psum slot (the WAR release that gates
            # this slot's next score group)
            cut = (gch + 1) // 2 if g % 2 == 0 else gch // 2
            lo = bass.ds(GOFF[g] * NSUP, cut * NSUP)
            hi = bass.ds((GOFF[g] + cut) * NSUP, (gch - cut) * NSUP)
            if cut:
                nc.scalar.activation(
                    e_sb[:, lo], ps[:, 0 : cut * NSUP],
                    mybir.ActivationFunctionType.Exp)
            if gch - cut:
                nc.vector.tensor_scalar(
                    e_sb[:, hi].bitcast(I16),
                    ps[:, bass.ds(cut * NSUP, (gch - cut) * NSUP)], SCH_MUL,
                    SCH_ADD, mybir.AluOpType.mult, mybir.AluOpType.add)

        def emit_vt_chunk(mi):
            pv = potile(f"pv{mi}")
            nc.tensor.matmul(pv[:, 0:VN], proj[:, bass.ts(mi, MCH)], vwb[:])
            if mi % 2 == 0:
                nc.vector.tensor_copy(vt[:, bass.ts(mi, VN)], pv[:, 0:VN])
            else:
                nc.scalar.copy(vt[:, bass.ts(mi, VN)], pv[:, 0:VN])

        def emit_block_epilogue(po, nbg):
            rcol = out_pool.tile([128, 1], F32, tag="rcol",
                                 name=f"rcol_{nbg}")
            nc.vector.reciprocal(rcol[:], po[:, C : C + 1])
            # fused (po * rcol) + xT in one DVE pass
            osb = out_pool.tile([128, C], F32, tag="osb", name=f"osb_{nbg}")
            nc.vector.scalar_tensor_tensor(
                osb[:], po[:, 0:C], rcol[:], xT[:, nbg, :],
                mybir.AluOpType.mult, mybir.AluOpType.add)
            nc.sync.dma_start(d_outT[:, nbg, :], osb[:])

        def emit_attnout_half(po, e_sb, nb, half):
            for mi in range(half * 16, half * 16 + 16):
                nc.tensor.matmul(
                    po[:, 0:VN],
                    e_sb[:, bass.ds(mi * NSUP + nb * NBLK, NBLK)],
                    vt[:, bass.ts(mi, VN)],
                    start=(mi == 0), stop=(mi == N_MCH - 1),
                )

        def emit_attnout_pair(ns, nbs):
            # interleave two blocks' accumulation chains chunk-by-chunk
            e_sb = e_sbs[ns]
            pos = [potile(f"pot_{ns}_{nb}") for nb in nbs]
            for mi in range(N_MCH):
                for po, nb in zip(pos, nbs):
                    nc.tensor.matmul(
                        po[:, 0:VN],
                        e_sb[:, bass.ds(mi * NSUP + nb * NBLK, NBLK)],
                        vt[:, bass.ts(mi, VN)],
                        start=(mi == 0), stop=(mi == N_MCH - 1),
                    )
            for po, nb in zip(pos, nbs):
                emit_block_epilogue(po, ns * n_blk + nb)

        # super 0: scores + exp interleaved with the vT' build. vt chunk
        # matmuls ride between score groups; copies alternate DVE/ACT. The
        # vt chunks share the po pool, which conveniently throttles the
        # first attnout block until the vT' build has drained.
        alloc_e(0)
        vt_done = 0
        for g in range(n_grp):
            emit_score_group(0, g)
            n_vt = 3 if g < 10 else 2
            for _ in range(n_vt):
                if vt_done < N_MCH:
                    emit_vt_chunk(vt_done)
                    vt_done += 1
        while vt_done < N_MCH:
            emit_vt_chunk(vt_done)
            vt_done += 1

        # steady state: per attnout block of super S, ~3 score groups of
        # super S+1 are emitted BETWEEN the two halves of the block's
        # matmul chain, so each group's exp (ACT or DVE, ~1.6-1.9us) has a
        # half-block (~1.8us) of PE work to drain before its psum slot
        # comes around again. The final super has no successor scores; its
        # blocks run as interleaved pairs.
        GSLOT = [[0, 1, 2], [3, 4, 5], [6, 7, 8], [9, 10]]
        for ns in range(n_sup):
            if ns + 1 < n_sup:
                alloc_e(ns + 1)
                for nb in range(n_blk):
                    e_sb = e_sbs[ns]
                    po = potile(f"po_{ns}_{nb}")
                    emit_attnout_half(po, e_sb, nb, 0)
                    emit_attnout_half(po, e_sb, nb, 1)
                    for g in GSLOT[nb]:
                        emit_score_group(ns + 1, g)
                    emit_block_epilogue(po, ns * n_blk + nb)
            else:
                emit_attnout_pair(ns, [0, 1])
                emit_attnout_pair(ns, [2, 3])
            e_sbs.pop(ns)

    nc.compile()
    return nc


def _prep_in_maps(x, conv_w, conv_b, q_w, q_b, k_w, k_b, v_w, v_b, gamma):
    g = np.float32(gamma[0])
    cwT = np.ascontiguousarray(conv_w.T.reshape(2, 128, C8)).astype(np.float16)
    kwT = np.concatenate([k_w.T, k_b[None, :]], axis=0).astype(np.float16)
    qwT = np.concatenate([q_w.T, q_b[None, :]], axis=0).astype(np.float16)
    vwb = np.zeros((C8 + 1, VN), np.float16)
    vwb[0:C8, 0:C] = (g * v_w).T.astype(np.float16)
    vwb[C8, 0:C] = (g * v_b).astype(np.float16)
    vwb[C8, C] = 1.0
    cb = conv_b.reshape(C8, 1).astype(np.float32)

    in_maps = []
    for core in range(8):
        b, hf = core // 2, core % 2
        xf = np.asarray(x[b], np.float32).reshape(C, HW)
        if hf:
            # rotate spatial columns: this core's query half -> cols 0:2048
            xf = np.roll(xf, -NQ, axis=1)
        # xT packed partition-major: [128 p, 16 nb, 256 c]
        xTp = np.ascontiguousarray(
            xf[:, 0:NQ].T.reshape(NB, 128, C).transpose(1, 0, 2))
        in_maps.append({
            "x16": np.ascontiguousarray(xf.reshape(2, 128, HW)).astype(
                np.float16),
            "xT": xTp,
            "cwT": cwT, "cb": cb, "kwT": kwT, "qwT": qwT, "vwb": vwb,
        })
    return in_maps


def kernel(x, conv_w, conv_b, q_w, q_b, k_w, k_b, v_w, v_b, gamma, **run_kw):
    if "nc" not in _CACHED:
        _CACHED["nc"] = build_nc()
    nc = _CACHED["nc"]
    in_maps = _prep_in_maps(x, conv_w, conv_b, q_w, q_b, k_w, k_b, v_w, v_b,
                            gamma)
    res = run_bass_kernel_spmd(nc, in_maps, core_ids=list(range(8)), **run_kw)
    _CACHED["last_result"] = res
    out = np.empty((B, C, HW), np.float32)
    for core in range(8):
        b, hf = core // 2, core % 2
        oc = np.asarray(res.results[core]["outT"])  # [128, 16, 256]
        out[b, :, hf * NQ : (hf + 1) * NQ] = \
            oc.transpose(1, 0, 2).reshape(NQ, C).T
    return out.reshape(B, C, H, W)
